# revision 5
# baseline (speedup 1.0000x reference)
"""Trainium2 8-core kernel for nn_MultiHeadAttention_83408264889124.

Full inputs in, full output out. Sharding: batch (4) x head-group (2) grid
over 8 NeuronCores — each core computes one batch with 6 of the 12 heads and
produces a partial Y^T = sum_h W_o[h]^T @ O_h^T; the host adds the two
head-group partials per batch (the "all-reduce" of the TP split) and
transposes back. All device work is in transposed layouts so no on-device
transposes are needed:

  Qt = (wq/sqrt(K))^T X^T, Kt = wk^T X^T          [K, S] per head
  St[k,q] = sum_d Kt[d,k] Qt[d,q]                  (2-head row-packed matmuls)
  E = exp(St)  (ScalarE, PSUM->SBUF bf16)
  AV with stationary [V_h | ones*64]: U[0:64] = V^T E, U[64:128] = colsum(E)
  Ot = U[0:64] * recip(U[64:128])                  (VectorE)

The reference does a RAW reshape [B,H,S,V] -> [B,S,H*V] (no transpose) before
W_o, which scrambles (head, seq): with t = S*h + s, output row s' = t//H gets
feature block j = t%H from head h, position s. Because S*HPC/H = 1024 exactly,
each head-group core produces a clean half of the output rows, and the scatter
indices depend only on the LOCAL head index - so one SPMD program works for
all cores. The output projection is Y^T = W_o^T @ G^T with the FULL W_o.

Schedule (the perf-critical part): the ScalarE exp stream is the bottleneck
resource (192 ACTIVATEs x ~1.1us with zero slack). The kernel therefore
minimizes time outside the exp-saturated window:
  - DMA order: wk, xk, wv, xv, wq, xq[c0], xq[c1:], wo; all DRAM tensors are
    host-packed partition-major ([128, ...] with large contiguous runs) so
    DMA runs at full rate.
  - Prologue (overlapped with DMA): PE warm-up burst, K-proj pair0 chunk-by-
    chunk behind the xk chunk DMAs, V-proj all kt behind xv, Q-proj pair0
    chunk0. First exp issues ~27us in.
  - All remaining projections (K/Q pairs 1-2, Q pair0 chunks 1-3) and the
    first output-projection column chunk (W_o c0 depends only on heads 0-2 =
    pairs 0,1) are emitted as single-matmul micro-units interleaved ~2 per kt
    iteration into the attention loops, filling the PE slack under the exp
    stream. Only W_o c1 (+ its Y DMA) remains as tail.
  - Attention-phase filler accumulations use a dedicated 1-bank PSUM pool
    (upool shrunk to 3 bufs to free the bank) so they never collide with the
    psS QK^T/exp rotation.

mask is all-ones for this problem (spec fill="ones") and adds 0 to logits, so
it is not read. Compute dtype bf16 (inputs converted host-side), f32
accumulation; softmax without max-subtraction (logits are O(1) by
construction so exp never overflows).
"""

from contextlib import ExitStack

import numpy as np
import ml_dtypes

import concourse.bacc as bacc
import concourse.bass as bass
import concourse.mybir as mybir
import concourse.tile as tile
from concourse.bass_utils import run_bass_kernel_spmd

BF16 = mybir.dt.bfloat16
F32 = mybir.dt.float32
I32 = mybir.dt.int32
EXP = mybir.ActivationFunctionType.Exp
RECIP_MAGIC = 0x7EF311C3

B, S, D, H, K, V = 4, 2048, 768, 12, 64, 64
HPC = 6  # heads per core
CH = 512  # q chunk


def build_nc(S=S, D=D, HPC=HPC, K=K, CH=CH):
    """Build the per-core Bass program (SPMD: same program on all 8 cores)."""
    assert D % 128 == 0 and S % 128 == 0 and S % CH == 0 and K == 64
    DT = D // 128  # contraction tiles for projections
    KT = S // 128  # key-position tiles
    QC = S // CH  # q chunks
    NP = HPC // 2  # head pairs
    FW = HPC * K  # per-core projection feature width (384)
    assert FW // 128 == NP

    nc = bacc.Bacc("TRN2", target_bir_lowering=False, debug=False, num_devices=8)

    # all DRAM tensors are partition-major: [128, ...] with the row-block
    # index folded into the column axis (host packs them; see _prep_in_maps)
    xq = nc.declare_dram_parameter("xq", [128, DT * S], BF16, isOutput=False)
    xk = nc.declare_dram_parameter("xk", [128, DT * S], BF16, isOutput=False)
    xv = nc.declare_dram_parameter("xv", [128, DT * S], BF16, isOutput=False)
    wq = nc.declare_dram_parameter("wq", [128, DT * FW], BF16, isOutput=False)
    wk = nc.declare_dram_parameter("wk", [128, DT * FW], BF16, isOutput=False)
    wv = nc.declare_dram_parameter("wv", [128, DT * FW], BF16, isOutput=False)
    wo = nc.declare_dram_parameter("wo", [128, DT * D], BF16, isOutput=False)
    SOUT = S * HPC // H  # output rows produced by this core (1024)
    y = nc.declare_dram_parameter("y", [128, DT * SOUT], F32, isOutput=True)

    with tile.TileContext(nc) as tc, ExitStack() as ctx:
        xpool = ctx.enter_context(tc.tile_pool(name="xin", bufs=1))
        wpool = ctx.enter_context(tc.tile_pool(name="w", bufs=1))
        qkpool = ctx.enter_context(tc.tile_pool(name="qk", bufs=1))
        vpool = ctx.enter_context(tc.tile_pool(name="vhat", bufs=1))
        opool = ctx.enter_context(tc.tile_pool(name="ot", bufs=1))
        epool = ctx.enter_context(tc.tile_pool(name="exps", bufs=10))
        rpool = ctx.enter_context(tc.tile_pool(name="rec", bufs=6))
        ypool = ctx.enter_context(tc.tile_pool(name="yev", bufs=4))
        # PSUM budget (8 banks): psS 2x[128,1024] (4) + U 3x[128,512] (3)
        # + 1-bank filler accumulator apool (1) = 8. The prologue projections
        # (before the attention rotation starts) borrow psS "s" slots.
        pspool = ctx.enter_context(tc.tile_pool(name="ps", bufs=2, space="PSUM"))
        upool = ctx.enter_context(tc.tile_pool(name="us", bufs=3, space="PSUM"))
        apool = ctx.enter_context(tc.tile_pool(name="acc", bufs=1, space="PSUM"))

        def load_w(dram):
            """DRAM [128, n] (partition-major) -> SBUF tile [128, n]."""
            t = wpool.tile(
                [128, dram.shape[1]], BF16, tag=dram.name, name=dram.name + "_sb"
            )
            nc.sync.dma_start(t[:], dram[:, :])
            return t

        xq_sb = xpool.tile([128, DT * S], BF16, tag="xq")
        xk_sb = xpool.tile([128, DT * S], BF16, tag="xk")
        xv_sb = xpool.tile([128, DT * S], BF16, tag="xv")

        def load_x_chunk(t, dram, qc):
            # dram x layout: [128, (qc, n, m)] -- chunk qc is one contiguous
            # [128, DT*CH] block (6 KiB runs per partition)
            t3 = t[:].rearrange("p (n m) -> p n m", m=S)
            d4 = dram[:, :].rearrange("p (q n m) -> p q n m", q=QC, m=CH)
            nc.sync.dma_start(
                t3[:, :, qc * CH : qc * CH + CH],
                d4[:, qc],
            )

        # DMA issue order = the prologue critical path: K proj -> V proj ->
        # Q chunk 0 -> attention; late xq chunks and wo are filler-phase deps
        wk_sb = load_w(wk)
        for qc in range(QC):
            load_x_chunk(xk_sb, xk, qc)
        wv_sb = load_w(wv)
        for qc in range(QC):
            load_x_chunk(xv_sb, xv, qc)
        wq_sb = load_w(wq)
        load_x_chunk(xq_sb, xq, 0)
        for qc in range(1, QC):
            load_x_chunk(xq_sb, xq, qc)
        wo_sb = load_w(wo)

        # PE warm-up burst: dependency-free matmuls that run during the
        # initial DMA wait so the HAM clock gate is at 8/8 (2.4 GHz) when
        # the first projection matmuls issue
        wu = wpool.tile([128, 128], BF16, tag="warm", name="warm")
        nc.vector.memset(wu[:], 0.0)
        pswu = pspool.tile([128, 2 * CH], F32, tag="s", name="pswu")
        for _ in range(56):
            nc.tensor.matmul(pswu[:, 0:128], wu[:], wu[:], start=True, stop=True)

        qt_sb = [
            qkpool.tile([128, S], BF16, tag=f"qt{p}", name=f"qt{p}")
            for p in range(NP)
        ]
        kt_sb = [
            qkpool.tile([128, S], BF16, tag=f"kt{p}", name=f"kt{p}")
            for p in range(NP)
        ]
        # G^T tiles: row 64j+v, col c — g-th tile holds j in {2g, 2g+1}
        gt_sb = [
            opool.tile([128, SOUT], BF16, tag=f"gt{g}", name=f"gt{g}")
            for g in range(D // 128)
        ]
        # vhat[kt]: [128, HPC*128]; head h occupies cols [128h,128h+128) as
        # [V_h (64) | ones (64)] — the ones columns make the AV matmul also
        # produce sum(exp) replicated across partitions 64..127.
        vhat = [
            vpool.tile([128, HPC * 128], BF16, tag=f"vh{k}", name=f"vh{k}")
            for k in range(KT)
        ]

        # Projection / output-projection work is emitted as micro-units
        # (one matmul or one eviction per unit) so it can be interleaved
        # into the attention kt-loop: the PE queue is in-order, so coarse
        # blocks would head-of-line-block the QK^T matmuls that feed the
        # (bottleneck) ScalarE exp stream.
        def acc_group_units(mm_emit, fin_emit, pool_tag, pslice=None):
            st = {}
            pool = pspool if pool_tag == "s" else apool

            def mk(dt):
                def f():
                    if dt == 0:
                        width = 2 * CH if pool_tag == "s" else CH
                        ps = pool.tile([128, width], F32, tag=pool_tag, name="acc")
                        st["ps"] = ps[:, : pslice or CH]
                    mm_emit(st["ps"], dt)

                return f

            return [mk(dt) for dt in range(DT)] + [lambda: fin_emit(st["ps"])]

        def proj_chunk_units(w_sb, x_sb, dst, hp, qc, pool_tag):
            def mm(ps, dt):
                nc.tensor.matmul(
                    ps[:],
                    w_sb[:, dt * FW + hp * 128 : dt * FW + hp * 128 + 128],
                    x_sb[:, dt * S + qc * CH : dt * S + qc * CH + CH],
                    start=(dt == 0),
                    stop=(dt == DT - 1),
                )

            def fin(ps):
                nc.vector.tensor_copy(dst[:, qc * CH : qc * CH + CH], ps[:])

            return acc_group_units(mm, fin, pool_tag)

        def v_proj_units(kt):
            # V projection for ALL heads at once (moving N=FW=384);
            # prologue-only, borrows psS "s" slots
            def mm(pv, dt):
                nc.tensor.matmul(
                    pv[:],
                    xv_sb[:, dt * S + kt * 128 : dt * S + kt * 128 + 128],
                    wv_sb[:, dt * FW : dt * FW + FW],
                    start=(dt == 0),
                    stop=(dt == DT - 1),
                )

            def fin(pv):
                dst3 = vhat[kt][:].rearrange("p (h m) -> p h m", m=128)
                nc.vector.tensor_copy(
                    dst3[:, :, 0:64],
                    pv[:].rearrange("p (h m) -> p h m", m=64),
                )
                nc.vector.memset(dst3[:, :, 64:128], 1.0)

            return acc_group_units(mm, fin, "s", pslice=FW)

        GT = D // 128

        def _wo_dt_units(qc, dt, pool_tag):
            def mm(py, g):
                nc.tensor.matmul(
                    py[:],
                    wo_sb[:, g * D + dt * 128 : g * D + dt * 128 + 128],
                    gt_sb[g][:, qc * CH : qc * CH + CH],
                    start=(g == 0),
                    stop=(g == GT - 1),
                )

            def fin(py):
                yt = ypool.tile([128, CH], F32, tag="yev", name="yt")
                nc.vector.tensor_copy(yt[:], py[:])
                nc.sync.dma_start(
                    y[:, dt * SOUT + qc * CH : dt * SOUT + qc * CH + CH],
                    yt[:],
                )

            st = {}
            pool = pspool if pool_tag == "s" else apool

            def mkmm(g):
                def f():
                    if g == 0:
                        width = 2 * CH if pool_tag == "s" else CH
                        ps = pool.tile([128, width], F32, tag=pool_tag, name="acc")
                        st["ps"] = ps[:, :CH]
                    mm(st["ps"], g)

                return f

            return [mkmm(g) for g in range(GT)] + [lambda: fin(st["ps"])]

        def wo_units(qc, dts, pool_tag):
            units = []
            for dt in dts:
                units += _wo_dt_units(qc, dt, pool_tag)
            return units

        # ---- prologue: shadowed by the input DMAs ----
        # K proj pair0 chunk-by-chunk behind the xk chunk loads
        for qc in range(QC):
            for un in proj_chunk_units(wk_sb, xk_sb, kt_sb[0], 0, qc, "s"):
                un()
        # V proj all kt behind the xv loads
        for kt in range(KT):
            for un in v_proj_units(kt):
                un()
        # Q proj pair0 chunk0
        for un in proj_chunk_units(wq_sb, xq_sb, qt_sb[0], 0, 0, "s"):
            un()

        # ---- filler queue: everything else, deadline-ordered ----
        fillers = []
        fillers += proj_chunk_units(wq_sb, xq_sb, qt_sb[0], 0, 1, "a")
        fillers += proj_chunk_units(wq_sb, xq_sb, qt_sb[0], 0, 2, "a")
        fillers += proj_chunk_units(wq_sb, xq_sb, qt_sb[0], 0, 3, "a")
        for qc in range(QC):
            fillers += proj_chunk_units(wk_sb, xk_sb, kt_sb[1], 1, qc, "a")
        for qc in range(QC):
            fillers += proj_chunk_units(wq_sb, xq_sb, qt_sb[1], 1, qc, "a")
        for qc in range(QC):
            fillers += proj_chunk_units(wk_sb, xk_sb, kt_sb[2], 2, qc, "a")
        for qc in range(QC):
            fillers += proj_chunk_units(wq_sb, xq_sb, qt_sb[2], 2, qc, "a")
        fillers.reverse()  # consume via pop()

        late_fillers = []  # W_o c0: legal only once pairs 0,1 scatters exist

        def pop_fillers(n):
            for _ in range(n):
                if fillers:
                    fillers.pop()()
                elif late_fillers:
                    late_fillers.pop()()

        for hp in range(NP):
            if hp == 2:
                # W_o col chunk 0 needs heads 0-2 only (= pairs 0,1): emit as
                # fillers under pair 2's attention
                late_fillers.extend(wo_units(0, range(GT), "a"))
                late_fillers.reverse()
            # attention for heads (2hp, 2hp+1); filler micro-units are
            # interleaved at matmul granularity into the kt loop so the PE
            # stream has work while ACT (exp) is the bottleneck
            for qc in range(QC):
                u_a = upool.tile([128, CH], F32, tag="u", name="ua")
                u_b = upool.tile([128, CH], F32, tag="u", name="ub")
                for kt in range(KT):
                    psS = pspool.tile([128, 2 * CH], F32, tag="s", name="psS")
                    # row-packed pair: head A rows 0-63, head B rows 64-127
                    nc.tensor.matmul(
                        psS[:, 0:CH],
                        kt_sb[hp][0:64, kt * 128 : kt * 128 + 128],
                        qt_sb[hp][0:64, qc * CH : qc * CH + CH],
                        start=True,
                        stop=True,
                    )
                    nc.tensor.matmul(
                        psS[:, CH : 2 * CH],
                        kt_sb[hp][64:128, kt * 128 : kt * 128 + 128],
                        qt_sb[hp][64:128, qc * CH : qc * CH + CH],
                        start=True,
                        stop=True,
                    )
                    es = epool.tile([128, 2 * CH], BF16, tag="es", name="es")
                    nc.scalar.activation(es[:], psS[:], EXP)
                    nc.tensor.matmul(
                        u_a[:],
                        vhat[kt][:, 256 * hp : 256 * hp + 128],
                        es[:, 0:CH],
                        start=(kt == 0),
                        stop=(kt == KT - 1),
                    )
                    nc.tensor.matmul(
                        u_b[:],
                        vhat[kt][:, 256 * hp + 128 : 256 * hp + 256],
                        es[:, CH : 2 * CH],
                        start=(kt == 0),
                        stop=(kt == KT - 1),
                    )
                    pop_fillers(2)
                for u, hl in ((u_a, 2 * hp), (u_b, 2 * hp + 1)):
                    # Newton reciprocal of the replicated exp-sums in rows
                    # 64..127 (magic-constant seed + 2 NR passes; w holds
                    # -1/l at ~1e-5 rel err)
                    den_i = u[64:128, :].bitcast(I32)
                    r = rpool.tile([64, CH], F32, tag="rec", name="r")
                    nc.vector.tensor_scalar(
                        r[:].bitcast(I32), den_i, RECIP_MAGIC, -1,
                        mybir.AluOpType.subtract, mybir.AluOpType.mult,
                    )
                    t = rpool.tile([64, CH], F32, tag="rec", name="t")
                    nc.vector.tensor_mul(t[:], u[64:128, :], r[:])
                    w = rpool.tile([64, CH], F32, tag="rec", name="w")
                    nc.vector.scalar_tensor_tensor(
                        w[:], t[:], 2.0, r[:],
                        mybir.AluOpType.subtract, mybir.AluOpType.mult,
                    )

                    # scatter-normalize: Ot[v, s] -> G^T[64j+v, c] with
                    # j=(S*hl+s)%H, c=(S*hl+s)//H; strided in s (step H);
                    # (u * -1) * w == u / l
                    cq0 = qc * CH
                    for j in range(H):
                        s0 = (j - S * hl) % H
                        m0 = max(0, -(-(cq0 - s0) // H))
                        s_st = s0 + H * m0
                        if s_st >= cq0 + CH:
                            continue
                        count = (cq0 + CH - 1 - s_st) // H + 1
                        o = s_st - cq0
                        c_st = (S * hl + s_st) // H
                        sl = slice(o, o + H * (count - 1) + 1, H)
                        nc.vector.scalar_tensor_tensor(
                            gt_sb[j // 2][
                                64 * (j % 2) : 64 * (j % 2) + 64,
                                c_st : c_st + count,
                            ],
                            u[0:64, sl],
                            -1.0,
                            w[:, sl],
                            mybir.AluOpType.mult,
                            mybir.AluOpType.mult,
                        )

        # drain any leftover fillers, then the W_o c1 tail
        while fillers or late_fillers:
            pop_fillers(1)
        for un in wo_units(1, range(GT), "a"):
            un()

    nc.compile()
    return nc


_NC_CACHE = None


def _get_nc():
    global _NC_CACHE
    if _NC_CACHE is None:
        _NC_CACHE = build_nc()
    return _NC_CACHE


def _pm_x(a):
    # [S, D] f32 -> X^T partition-major [128, QC, DT, CH] -> [128, DT*S]
    bf = ml_dtypes.bfloat16
    QC = S // CH
    DT = D // 128
    t = a.T.astype(bf)  # [D, S]
    return (
        t.reshape(DT, 128, QC, CH).transpose(1, 2, 0, 3).reshape(128, DT * S)
    )


def _pm_w(w):
    # [D, F] -> partition-major [128, DT*F]
    bf = ml_dtypes.bfloat16
    DT = D // 128
    F = w.shape[1]
    return w.astype(bf).reshape(DT, 128, F).transpose(1, 0, 2).reshape(128, DT * F)


def _prep_in_maps(queries, keys, values, W_q, W_k, W_v, W_o):
    scale = np.float32(1.0 / np.sqrt(K))
    in_maps = []
    xq_pm = [_pm_x(queries[b]) for b in range(B)]
    xk_pm = [_pm_x(keys[b]) for b in range(B)]
    xv_pm = [_pm_x(values[b]) for b in range(B)]
    wo_pm = _pm_w(W_o)  # full W_o: the raw-reshape scramble touches all rows
    for core in range(8):
        b, hg = divmod(core, 2)
        h0 = hg * HPC
        wq_c = (W_q[h0 : h0 + HPC] * scale).transpose(1, 0, 2).reshape(D, HPC * K)
        wk_c = W_k[h0 : h0 + HPC].transpose(1, 0, 2).reshape(D, HPC * K)
        wv_c = W_v[h0 : h0 + HPC].transpose(1, 0, 2).reshape(D, HPC * V)
        in_maps.append(
            {
                "xq": xq_pm[b],
                "xk": xk_pm[b],
                "xv": xv_pm[b],
                "wq": _pm_w(wq_c),
                "wk": _pm_w(wk_c),
                "wv": _pm_w(wv_c),
                "wo": wo_pm,
            }
        )
    return in_maps


def run(inputs, trace=False, **spmd_kwargs):
    """Run on 8 cores; returns (full_output [B,S,D] f32, BassKernelResults)."""
    queries = np.asarray(inputs["queries"], np.float32)
    keys = np.asarray(inputs["keys"], np.float32)
    values = np.asarray(inputs["values"], np.float32)
    W_q = np.asarray(inputs["W_q"], np.float32)
    W_k = np.asarray(inputs["W_k"], np.float32)
    W_v = np.asarray(inputs["W_v"], np.float32)
    W_o = np.asarray(inputs["W_o"], np.float32)

    nc = _get_nc()
    in_maps = _prep_in_maps(queries, keys, values, W_q, W_k, W_v, W_o)
    res = run_bass_kernel_spmd(
        nc, in_maps, core_ids=list(range(8)), trace=trace, **spmd_kwargs
    )
    out = np.empty((B, S, D), np.float32)
    half = S * HPC // H  # 1024 output rows per head-group core
    DT = D // 128
    for b in range(B):
        for hg in range(2):
            y_pm = res.results[2 * b + hg]["y"]  # [128, DT*half]
            yt = y_pm.reshape(128, DT, half).transpose(1, 0, 2).reshape(D, half)
            out[b, hg * half : (hg + 1) * half] = yt.T
    return out, res


def kernel(**inputs) -> np.ndarray:
    out, _ = run(inputs, trace=False)
    return out


# revision 10
# speedup vs baseline: 1.0769x; 1.0769x over previous
"""Trainium2 8-core kernel for nn_MultiHeadAttention_83408264889124.

Full inputs in, full output out. Sharding: batch (4) x head-group (2) grid
over 8 NeuronCores — each core computes one batch with 6 of the 12 heads and
produces a partial Y^T = sum_h W_o[h]^T @ O_h^T; the host adds the two
head-group partials per batch (the "all-reduce" of the TP split) and
transposes back. All device work is in transposed layouts so no on-device
transposes are needed:

  Qt = (wq/sqrt(K))^T X^T, Kt = wk^T X^T          [K, S] per head
  St[k,q] = sum_d Kt[d,k] Qt[d,q]                  (2-head row-packed matmuls)
  E = exp(St)  (ScalarE, PSUM->SBUF bf16)
  AV with stationary [V_h | ones*64]: U[0:64] = V^T E, U[64:128] = colsum(E)
  Ot = U[0:64] * recip(U[64:128])                  (VectorE)

The reference does a RAW reshape [B,H,S,V] -> [B,S,H*V] (no transpose) before
W_o, which scrambles (head, seq): with t = S*h + s, output row s' = t//H gets
feature block j = t%H from head h, position s. Because S*HPC/H = 1024 exactly,
each head-group core produces a clean half of the output rows, and the scatter
indices depend only on the LOCAL head index - so one SPMD program works for
all cores. The output projection is Y^T = W_o^T @ G^T with the FULL W_o.

Schedule (the perf-critical part): the ScalarE exp stream is the bottleneck
resource (192 ACTIVATEs x ~1.1us with zero slack). The kernel therefore
minimizes time outside the exp-saturated window:
  - DMA order: wk, xk, wv, xv, wq, xq[c0], xq[c1:], wo; all DRAM tensors are
    host-packed partition-major ([128, ...] with large contiguous runs) so
    DMA runs at full rate.
  - Prologue (overlapped with DMA): PE warm-up burst, K-proj pair0 chunk-by-
    chunk behind the xk chunk DMAs, V-proj all kt behind xv, Q-proj pair0
    chunk0. First exp issues ~27us in.
  - All remaining projections (K/Q pairs 1-2, Q pair0 chunks 1-3) and the
    first output-projection column chunk (W_o c0 depends only on heads 0-2 =
    pairs 0,1) are emitted as single-matmul micro-units interleaved ~2 per kt
    iteration into the attention loops, filling the PE slack under the exp
    stream. Only W_o c1 (+ its Y DMA) remains as tail.
  - Attention-phase filler accumulations use a dedicated 1-bank PSUM pool
    (upool shrunk to 3 bufs to free the bank) so they never collide with the
    psS QK^T/exp rotation.

mask is all-ones for this problem (spec fill="ones") and adds 0 to logits, so
it is not read. Compute dtype bf16 (inputs converted host-side), f32
accumulation; softmax without max-subtraction (logits are O(1) by
construction so exp never overflows).
"""

from contextlib import ExitStack

import numpy as np
import ml_dtypes

import concourse.bacc as bacc
import concourse.bass as bass
import concourse.mybir as mybir
import concourse.tile as tile
from concourse.bass_utils import run_bass_kernel_spmd

BF16 = mybir.dt.bfloat16
F32 = mybir.dt.float32
I32 = mybir.dt.int32
EXP = mybir.ActivationFunctionType.Exp
RECIP_MAGIC = 0x7EF311C3

B, S, D, H, K, V = 4, 2048, 768, 12, 64, 64
HPC = 6  # heads per core
CH = 512  # q chunk


def build_nc(S=S, D=D, HPC=HPC, K=K, CH=CH):
    """Build the per-core Bass program (SPMD: same program on all 8 cores)."""
    assert D % 128 == 0 and S % 128 == 0 and S % CH == 0 and K == 64
    DT = D // 128  # contraction tiles for projections
    KT = S // 128  # key-position tiles
    QC = S // CH  # q chunks
    NP = HPC // 2  # head pairs
    FW = HPC * K  # per-core projection feature width (384)
    assert FW // 128 == NP

    nc = bacc.Bacc("TRN2", target_bir_lowering=False, debug=False, num_devices=8)

    # all DRAM tensors are partition-major: [128, ...] with the row-block
    # index folded into the column axis (host packs them; see _prep_in_maps)
    xq = nc.declare_dram_parameter("xq", [128, DT * S], BF16, isOutput=False)
    xk = nc.declare_dram_parameter("xk", [128, DT * S], BF16, isOutput=False)
    xv = nc.declare_dram_parameter("xv", [128, DT * S], BF16, isOutput=False)
    wq = nc.declare_dram_parameter("wq", [128, DT * FW], BF16, isOutput=False)
    wk = nc.declare_dram_parameter("wk", [128, DT * FW], BF16, isOutput=False)
    wv = nc.declare_dram_parameter("wv", [128, DT * FW], BF16, isOutput=False)
    wo = nc.declare_dram_parameter("wo", [128, DT * D], BF16, isOutput=False)
    SOUT = S * HPC // H  # output rows produced by this core (1024)
    y = nc.declare_dram_parameter("y", [128, DT * SOUT], F32, isOutput=True)

    with tile.TileContext(nc) as tc, ExitStack() as ctx:
        xpool = ctx.enter_context(tc.tile_pool(name="xin", bufs=1))
        wpool = ctx.enter_context(tc.tile_pool(name="w", bufs=1))
        qkpool = ctx.enter_context(tc.tile_pool(name="qk", bufs=1))
        vpool = ctx.enter_context(tc.tile_pool(name="vhat", bufs=1))
        opool = ctx.enter_context(tc.tile_pool(name="ot", bufs=1))
        epool = ctx.enter_context(tc.tile_pool(name="exps", bufs=10))
        rpool = ctx.enter_context(tc.tile_pool(name="rec", bufs=4))
        unpool = ctx.enter_context(tc.tile_pool(name="un", bufs=4))
        ypool = ctx.enter_context(tc.tile_pool(name="yev", bufs=4))
        # PSUM budget (8 banks): psS 2x[128,1024] (4) + U 3x[128,512] (3)
        # + 1-bank filler accumulator apool (1) = 8. The prologue projections
        # (before the attention rotation starts) borrow psS "s" slots.
        pspool = ctx.enter_context(tc.tile_pool(name="ps", bufs=2, space="PSUM"))
        upool = ctx.enter_context(tc.tile_pool(name="us", bufs=3, space="PSUM"))
        apool = ctx.enter_context(tc.tile_pool(name="acc", bufs=1, space="PSUM"))

        def load_w(dram):
            """DRAM [128, n] (partition-major) -> SBUF tile [128, n]."""
            t = wpool.tile(
                [128, dram.shape[1]], BF16, tag=dram.name, name=dram.name + "_sb"
            )
            nc.sync.dma_start(t[:], dram[:, :])
            return t

        xq_sb = xpool.tile([128, DT * S], BF16, tag="xq")
        xk_sb = xpool.tile([128, DT * S], BF16, tag="xk")
        xv_sb = xpool.tile([128, DT * S], BF16, tag="xv")

        def load_x_chunk(t, dram, qc):
            # dram x layout: [128, (qc, n, m)] -- chunk qc is one contiguous
            # [128, DT*CH] block (6 KiB runs per partition)
            t3 = t[:].rearrange("p (n m) -> p n m", m=S)
            d4 = dram[:, :].rearrange("p (q n m) -> p q n m", q=QC, m=CH)
            nc.sync.dma_start(
                t3[:, :, qc * CH : qc * CH + CH],
                d4[:, qc],
            )

        # DMA issue order: xv first (V-proj is the bulk of prologue PE work
        # and runs under the later xk/xq DMAs), then the first-exp critical
        # path wk+xk -> wq+xq[c0,c1]; late xq chunks and wo are filler deps
        wv_sb = load_w(wv)
        for qc in range(QC):
            load_x_chunk(xv_sb, xv, qc)
        wk_sb = load_w(wk)
        for qc in range(QC):
            load_x_chunk(xk_sb, xk, qc)
        wq_sb = load_w(wq)
        for qc in range(QC):
            load_x_chunk(xq_sb, xq, qc)
        wo_sb = load_w(wo)

        # PE warm-up burst: dependency-free matmuls that run during the
        # initial DMA wait so the HAM clock gate is at 8/8 (2.4 GHz) when
        # the first projection matmuls issue
        wu = wpool.tile([128, 128], BF16, tag="warm", name="warm")
        nc.vector.memset(wu[:], 0.0)
        pswu = pspool.tile([128, 2 * CH], F32, tag="s", name="pswu")
        for _ in range(56):
            nc.tensor.matmul(pswu[:, 0:128], wu[:], wu[:], start=True, stop=True)

        qt_sb = [
            qkpool.tile([128, S], BF16, tag=f"qt{p}", name=f"qt{p}")
            for p in range(NP)
        ]
        kt_sb = [
            qkpool.tile([128, S], BF16, tag=f"kt{p}", name=f"kt{p}")
            for p in range(NP)
        ]
        # G^T tiles: row 64j+v, col c — g-th tile holds j in {2g, 2g+1}
        gt_sb = [
            opool.tile([128, SOUT], BF16, tag=f"gt{g}", name=f"gt{g}")
            for g in range(D // 128)
        ]
        # vhat[kt]: [128, HPC*128]; head h occupies cols [128h,128h+128) as
        # [V_h (64) | ones (64)] — the ones columns make the AV matmul also
        # produce sum(exp) replicated across partitions 64..127.
        vhat = [
            vpool.tile([128, HPC * 128], BF16, tag=f"vh{k}", name=f"vh{k}")
            for k in range(KT)
        ]

        # Projection / output-projection work is emitted as micro-units
        # (one matmul or one eviction per unit) so it can be interleaved
        # into the attention kt-loop: the PE queue is in-order, so coarse
        # blocks would head-of-line-block the QK^T matmuls that feed the
        # (bottleneck) ScalarE exp stream.
        def acc_group_units(mm_emit, fin_emit, pool_tag, pslice=None):
            st = {}
            pool = pspool if pool_tag == "s" else apool

            def mk(dt):
                def f():
                    if dt == 0:
                        width = 2 * CH if pool_tag == "s" else CH
                        ps = pool.tile([128, width], F32, tag=pool_tag, name="acc")
                        st["ps"] = ps[:, : pslice or CH]
                    mm_emit(st["ps"], dt)

                return f

            return [mk(dt) for dt in range(DT)] + [lambda: fin_emit(st["ps"])]

        def proj_chunk_units(w_sb, x_sb, dst, hp, qc, pool_tag):
            def mm(ps, dt):
                nc.tensor.matmul(
                    ps[:],
                    w_sb[:, dt * FW + hp * 128 : dt * FW + hp * 128 + 128],
                    x_sb[:, dt * S + qc * CH : dt * S + qc * CH + CH],
                    start=(dt == 0),
                    stop=(dt == DT - 1),
                )

            def fin(ps):
                nc.vector.tensor_copy(dst[:, qc * CH : qc * CH + CH], ps[:])

            return acc_group_units(mm, fin, pool_tag)

        def v_proj_units(kt, pool_tag="s"):
            # V projection for ALL heads at once (moving N=FW=384)
            def mm(pv, dt):
                nc.tensor.matmul(
                    pv[:],
                    xv_sb[:, dt * S + kt * 128 : dt * S + kt * 128 + 128],
                    wv_sb[:, dt * FW : dt * FW + FW],
                    start=(dt == 0),
                    stop=(dt == DT - 1),
                )

            def fin(pv):
                dst3 = vhat[kt][:].rearrange("p (h m) -> p h m", m=128)
                nc.vector.tensor_copy(
                    dst3[:, :, 0:64],
                    pv[:].rearrange("p (h m) -> p h m", m=64),
                )
                nc.vector.memset(dst3[:, :, 64:128], 1.0)

            return acc_group_units(mm, fin, pool_tag, pslice=FW)

        def v_proj_units_a(kt):
            return v_proj_units(kt, "a")

        GT = D // 128

        def _wo_dt_units(qc, dt, pool_tag):
            def mm(py, g):
                nc.tensor.matmul(
                    py[:],
                    wo_sb[:, g * D + dt * 128 : g * D + dt * 128 + 128],
                    gt_sb[g][:, qc * CH : qc * CH + CH],
                    start=(g == 0),
                    stop=(g == GT - 1),
                )

            def fin(py):
                yt = ypool.tile([128, CH], F32, tag="yev", name="yt")
                nc.vector.tensor_copy(yt[:], py[:])
                nc.sync.dma_start(
                    y[:, dt * SOUT + qc * CH : dt * SOUT + qc * CH + CH],
                    yt[:],
                )

            st = {}
            pool = pspool if pool_tag == "s" else apool

            def mkmm(g):
                def f():
                    if g == 0:
                        width = 2 * CH if pool_tag == "s" else CH
                        ps = pool.tile([128, width], F32, tag=pool_tag, name="acc")
                        st["ps"] = ps[:, :CH]
                    mm(st["ps"], g)

                return f

            return [mkmm(g) for g in range(GT)] + [lambda: fin(st["ps"])]

        def wo_units(qc, dts, pool_tag):
            units = []
            for dt in dts:
                units += _wo_dt_units(qc, dt, pool_tag)
            return units

        # ---- prologue: shadowed by the input DMAs ----
        # V proj kts 0-7 behind the xv loads (the rest go in as fillers)
        VPRE = 8
        for kt in range(VPRE):
            for un in v_proj_units(kt):
                un()
        # K proj pair0 chunk-by-chunk behind the xk chunk loads
        for qc in range(QC):
            for un in proj_chunk_units(wk_sb, xk_sb, kt_sb[0], 0, qc, "s"):
                un()
        # Q proj pair0 chunks 0-1 (chunk 1 is needed at the first qc switch,
        # too early for the filler stream)
        for un in proj_chunk_units(wq_sb, xq_sb, qt_sb[0], 0, 0, "s"):
            un()
        for un in proj_chunk_units(wq_sb, xq_sb, qt_sb[0], 0, 1, "s"):
            un()

        # ---- filler queue: everything else, deadline-ordered ----
        fillers = []
        n_vunits = 0
        for kt in range(VPRE, KT):
            vu = v_proj_units_a(kt)
            n_vunits += len(vu)
            fillers += vu
        fillers += proj_chunk_units(wq_sb, xq_sb, qt_sb[0], 0, 2, "a")
        fillers += proj_chunk_units(wq_sb, xq_sb, qt_sb[0], 0, 3, "a")
        for qc in range(QC):
            fillers += proj_chunk_units(wk_sb, xk_sb, kt_sb[1], 1, qc, "a")
        for qc in range(QC):
            fillers += proj_chunk_units(wq_sb, xq_sb, qt_sb[1], 1, qc, "a")
        for qc in range(QC):
            fillers += proj_chunk_units(wk_sb, xk_sb, kt_sb[2], 2, qc, "a")
        for qc in range(QC):
            fillers += proj_chunk_units(wq_sb, xq_sb, qt_sb[2], 2, qc, "a")
        n_rest = len(fillers) - n_vunits
        fillers.reverse()  # consume via pop()

        late_fillers = []  # W_o c0: legal only once pairs 0,1 scatters exist

        def pop_fillers(n):
            for _ in range(n):
                if fillers:
                    fillers.pop()()
                elif late_fillers:
                    late_fillers.pop()()

        def emit_qkt(hp, qc, kt):
            psS = pspool.tile([128, 2 * CH], F32, tag="s", name="psS")
            # row-packed pair: head A rows 0-63, head B rows 64-127; the two
            # matmuls land on disjoint PE row-groups and run concurrently
            nc.tensor.matmul(
                psS[:, 0:CH],
                kt_sb[hp][0:64, kt * 128 : kt * 128 + 128],
                qt_sb[hp][0:64, qc * CH : qc * CH + CH],
                start=True,
                stop=True,
            )
            nc.tensor.matmul(
                psS[:, CH : 2 * CH],
                kt_sb[hp][64:128, kt * 128 : kt * 128 + 128],
                qt_sb[hp][64:128, qc * CH : qc * CH + CH],
                start=True,
                stop=True,
            )
            return psS

        def emit_normalize(u_a, u_b, hp, qc):
            for u, hl in ((u_a, 2 * hp), (u_b, 2 * hp + 1)):
                # Newton reciprocal of the replicated exp-sums in rows
                # 64..127 (magic-constant seed + 2 NR passes; w holds
                # -1/l at ~1e-5 rel err)
                den_i = u[64:128, :].bitcast(I32)
                r = rpool.tile([64, CH], F32, tag="rec", name="r")
                nc.vector.tensor_scalar(
                    r[:].bitcast(I32), den_i, RECIP_MAGIC, -1,
                    mybir.AluOpType.subtract, mybir.AluOpType.mult,
                )
                t = rpool.tile([64, CH], F32, tag="rec", name="t")
                nc.vector.tensor_mul(t[:], u[64:128, :], r[:])
                w = rpool.tile([64, CH], F32, tag="rec", name="w")
                nc.vector.scalar_tensor_tensor(
                    w[:], t[:], 2.0, r[:],
                    mybir.AluOpType.subtract, mybir.AluOpType.mult,
                )
                # stage the normalized output to SBUF in ONE op so the PSUM
                # u slot frees fast (u pool is only 3 bufs deep); the gt
                # scatter then reads the staging tile off the critical path
                un = unpool.tile([64, CH], F32, tag="un", name="un")
                nc.vector.scalar_tensor_tensor(
                    un[:], u[0:64, :], -1.0, w[:],
                    mybir.AluOpType.mult, mybir.AluOpType.mult,
                )

                # scatter: Ot[v, s] -> G^T[64j+v, c] with j=(S*hl+s)%H,
                # c=(S*hl+s)//H; strided in s (step H)
                cq0 = qc * CH
                for j in range(H):
                    s0 = (j - S * hl) % H
                    m0 = max(0, -(-(cq0 - s0) // H))
                    s_st = s0 + H * m0
                    if s_st >= cq0 + CH:
                        continue
                    count = (cq0 + CH - 1 - s_st) // H + 1
                    o = s_st - cq0
                    c_st = (S * hl + s_st) // H
                    sl = slice(o, o + H * (count - 1) + 1, H)
                    nc.vector.tensor_copy(
                        gt_sb[j // 2][
                            64 * (j % 2) : 64 * (j % 2) + 64,
                            c_st : c_st + count,
                        ],
                        un[:, sl],
                    )

        # ---- attention: flat kt stream with one-slot QK^T lookahead so the
        # next qc's logits are already in flight when a qc ends (keeps the
        # exp stream gap-free across qc boundaries) ----
        slots = [
            (hp, qc, kt)
            for hp in range(NP)
            for qc in range(QC)
            for kt in range(KT)
        ]
        psS_cur = emit_qkt(*slots[0])
        u_a = u_b = None
        for g, (hp, qc, kt) in enumerate(slots):
            if hp == 2 and qc == 0 and kt == 0:
                # W_o col chunk 0 needs heads 0-2 only (= pairs 0,1): emit
                # as fillers under pair 2's attention
                late_fillers.extend(wo_units(0, range(GT), "a"))
                late_fillers.reverse()
            if kt == 0:
                u_a = upool.tile([128, CH], F32, tag="u", name="ua")
                u_b = upool.tile([128, CH], F32, tag="u", name="ub")
            es = epool.tile([128, 2 * CH], BF16, tag="es", name="es")
            nc.scalar.activation(es[:], psS_cur[:], EXP)
            if g + 1 < len(slots):
                psS_cur = emit_qkt(*slots[g + 1])
            nc.tensor.matmul(
                u_a[:],
                vhat[kt][:, 256 * hp : 256 * hp + 128],
                es[:, 0:CH],
                start=(kt == 0),
                stop=(kt == KT - 1),
            )
            nc.tensor.matmul(
                u_b[:],
                vhat[kt][:, 256 * hp + 128 : 256 * hp + 256],
                es[:, CH : 2 * CH],
                start=(kt == 0),
                stop=(kt == KT - 1),
            )
            if kt == KT - 1:
                emit_normalize(u_a, u_b, hp, qc)
            # V-proj fillers must stay ahead of the AVs that consume them
            pop_fillers(4 if len(fillers) > n_rest else 2)

        # drain any leftover fillers, then the W_o c1 tail (psS "s" slots
        # are free now -> double-buffered accumulation, no apool WAR chain)
        while fillers or late_fillers:
            pop_fillers(1)
        for un in wo_units(1, range(GT), "s"):
            un()

    nc.compile()
    return nc


_NC_CACHE = None


def _get_nc():
    global _NC_CACHE
    if _NC_CACHE is None:
        _NC_CACHE = build_nc()
    return _NC_CACHE


def _pm_x(a):
    # [S, D] f32 -> X^T partition-major [128, QC, DT, CH] -> [128, DT*S]
    bf = ml_dtypes.bfloat16
    QC = S // CH
    DT = D // 128
    t = a.T.astype(bf)  # [D, S]
    return (
        t.reshape(DT, 128, QC, CH).transpose(1, 2, 0, 3).reshape(128, DT * S)
    )


def _pm_w(w):
    # [D, F] -> partition-major [128, DT*F]
    bf = ml_dtypes.bfloat16
    DT = D // 128
    F = w.shape[1]
    return w.astype(bf).reshape(DT, 128, F).transpose(1, 0, 2).reshape(128, DT * F)


def _prep_in_maps(queries, keys, values, W_q, W_k, W_v, W_o):
    scale = np.float32(1.0 / np.sqrt(K))
    in_maps = []
    xq_pm = [_pm_x(queries[b]) for b in range(B)]
    xk_pm = [_pm_x(keys[b]) for b in range(B)]
    xv_pm = [_pm_x(values[b]) for b in range(B)]
    wo_pm = _pm_w(W_o)  # full W_o: the raw-reshape scramble touches all rows
    for core in range(8):
        b, hg = divmod(core, 2)
        h0 = hg * HPC
        wq_c = (W_q[h0 : h0 + HPC] * scale).transpose(1, 0, 2).reshape(D, HPC * K)
        wk_c = W_k[h0 : h0 + HPC].transpose(1, 0, 2).reshape(D, HPC * K)
        wv_c = W_v[h0 : h0 + HPC].transpose(1, 0, 2).reshape(D, HPC * V)
        in_maps.append(
            {
                "xq": xq_pm[b],
                "xk": xk_pm[b],
                "xv": xv_pm[b],
                "wq": _pm_w(wq_c),
                "wk": _pm_w(wk_c),
                "wv": _pm_w(wv_c),
                "wo": wo_pm,
            }
        )
    return in_maps


def run(inputs, trace=False, **spmd_kwargs):
    """Run on 8 cores; returns (full_output [B,S,D] f32, BassKernelResults)."""
    queries = np.asarray(inputs["queries"], np.float32)
    keys = np.asarray(inputs["keys"], np.float32)
    values = np.asarray(inputs["values"], np.float32)
    W_q = np.asarray(inputs["W_q"], np.float32)
    W_k = np.asarray(inputs["W_k"], np.float32)
    W_v = np.asarray(inputs["W_v"], np.float32)
    W_o = np.asarray(inputs["W_o"], np.float32)

    nc = _get_nc()
    in_maps = _prep_in_maps(queries, keys, values, W_q, W_k, W_v, W_o)
    res = run_bass_kernel_spmd(
        nc, in_maps, core_ids=list(range(8)), trace=trace, **spmd_kwargs
    )
    out = np.empty((B, S, D), np.float32)
    half = S * HPC // H  # 1024 output rows per head-group core
    DT = D // 128
    for b in range(B):
        for hg in range(2):
            y_pm = res.results[2 * b + hg]["y"]  # [128, DT*half]
            yt = y_pm.reshape(128, DT, half).transpose(1, 0, 2).reshape(D, half)
            out[b, hg * half : (hg + 1) * half] = yt.T
    return out, res


def kernel(**inputs) -> np.ndarray:
    out, _ = run(inputs, trace=False)
    return out
